# revision 34
# baseline (speedup 1.0000x reference)
"""Trainium2 Bass kernel for nn_AbsoluteMinimalBlock (rmsnorm -> rank-1 SSM scan -> rmsnorm -> rank-2 FFN).

Math: the whole block is a rank-3 update of x:
    out[t,d] = x[t,d] + h[t]*Wout[d] + g0[t]*W20[d] + g1[t]*W21[d]
  driven by 5 per-token reductions over D:
    d1 = x@(nw*W_in), dW2 = x@(2*Wout/D), dA = x@(nw*w1_0), dB = x@(nw*w1_1),
    S0 = sum(x^2)
  with v = S0/D+eps; rstd1 = rsqrt(v); u = d1*rstd1; h = scan(a, u);
  ms2 = v + h*dW2 + h^2*cWW/D (analytic); rstd2 = rsqrt(ms2);
  p_r = (d_r + h*(Wout.W1r))*rstd2; g_r = gelu_tanh(p_r).

v4 design (group-pipelined, all I/O bf16, DVE fast modes):
  - 9 groups of <=4 tiles, software-pipelined with explicit stage lags so
    every cross-engine dependency is >=1 iteration old (the 4-deep engine
    wait queues otherwise serialize): A(load/square/transpose/copy) ->
    dots(lag1) -> scan(lag2) -> gelu(lag3) -> ctT(lag4) -> w3+add+store(lag5).
  - dots as stationary-x matmuls: d4[tok,4] += pair_sb_chunk.T @ vw4_chunk
    writes the per-token reductions directly in token-partition layout
    (no [4,512] intermediate, no second transpose).
  - rank-3 reconstruct: single K=3 matmul into a BF16 PSUM tile, then one
    DVE tensor_add (bf16 2x mode) + residual; no ACT copies, no identity
    matmuls.
  - ACT does the squares (Square+accum) and small copies via int32-bitcast
    views (halves ACT cycles); DVE does PSUM->SBUF pair copies (bf16 2x)
    and the adds; Pool does the rstd2/gelu-arg scalar chain; per-token
    scalars live in [128tok, ntile] layout so ops are partition-parallel.
  - rsqrt WITHOUT the Scalar-engine Sqrt (sqrt and gelu live in different
    ACT table sets -> 1.3us reload per switch): quake bit-trick seed +
    one Newton step on DVE int ops; rstd2 by one Newton step seeded
    from rstd1. ACT only ever needs {Square, Copy, Gelu_apprx_tanh}.
  - constants DMA on the ACT queue so x in-DMAs start immediately on SP;
    out-DMAs issue from the DVE queue right after the pair's adds.

Sharding: 8 cores = 4 batches x 2 sequence halves; each core prepends a
PRE-tile prefix (zeros for first half / tail of first half for second) so the
scan state is exact (a^128 per tile of decay).
"""
import sys, os
DOTS_LAG = int(os.environ.get("DOTS_LAG", "0"))
SQ_DVE_N = int(os.environ.get("SQ_DVE_N", "3"))      # squares on DVE (of 33)
C_ACT_N = int(os.environ.get("C_ACT_N", "16"))       # residuals on ACT (of 32)
OUT_QUAD = int(os.environ.get("OUT_QUAD", "0"))      # 1 -> 4-tile out DMAs
PAIRCP_ACT_N = int(os.environ.get("PAIRCP_ACT_N", "0"))  # pair copies on ACT (of 40)
SBLOCK_W = int(os.environ.get("SBLOCK_W", "2"))      # groups per scalar sblock
for _p in ("/root/.axon_site/_ro/trn_rl_repo", "/opt/trn_rl_repo"):
    if os.path.isdir(_p) and _p not in sys.path:
        sys.path.append(_p)

import numpy as np
import ml_dtypes
import concourse.bass as bass
import concourse.bacc as bacc
import concourse.mybir as mybir
import concourse.tile as tile
from concourse.bass_utils import run_bass_kernel_spmd

F32 = mybir.dt.float32
I32 = mybir.dt.int32
BF16 = mybir.dt.bfloat16
AF = mybir.ActivationFunctionType
OP = mybir.AluOpType
BF_NP = ml_dtypes.bfloat16

N_CORES = 8
B, S, D = 4, 8192, 1024
HALF = S // 2
MAIN_TILES = HALF // 128      # 32
EPS = 1e-6
MAGIC = 0x5F3759DF

# engine-balance knobs (tuned against the timeline sim + hardware):
# pair copies with (idx % 5) < PAIRCP_ACT_LT run on ACT (int32 view), rest DVE
PAIRCP_ACT_LT = 0
# squares on DVE for tiles with i % 16 == SQ_DVE_MOD, rest ACT
SQ_DVE_MOD = 5  # i % 11 == 5 -> DVE
# residual: tiles with i % 3 == C_ACT_MOD use PE-ident+ACT-copy, rest DVE add
C_ACT_MOD = 1
C_ACT_PERIOD = 2

_cache = {}


def group_bounds(nt):
    """Tile groups aligned to out-DMA pairs: [0,1,2], 4s, then 2s at the
    tail so the pipeline drain processes small groups."""
    gs = [(0, 3)]
    t = 3
    while t + 4 <= nt - 6:
        gs.append((t, t + 4))
        t += 4
    while t < nt:
        gs.append((t, t + 2))
        t += 2
    return gs


def build_program(nt: int, reps=None, internal_io=False, parts=None, unroll=1):
    pre_tiles = nt - MAIN_TILES
    assert pre_tiles == 1, "pair layout assumes exactly one prefix tile"
    groups = group_bounds(nt)
    G = len(groups)

    nc = bacc.Bacc("TRN2", target_bir_lowering=False, debug=False, num_devices=N_CORES)

    if internal_io:
        xd = nc.dram_tensor("x_int", [nt * 128, D], BF16, kind="Internal").ap()
        yd = nc.dram_tensor("y_int", [HALF, D], BF16, kind="Internal").ap()
        dummy_in = nc.dram_tensor("x_in", [128, 4], F32, kind="ExternalInput").ap()
        dummy_out = nc.dram_tensor("y_out", [128, 4], F32, kind="ExternalOutput").ap()
        need_dummy_io = True
    else:
        xd = nc.dram_tensor("x_in", [nt * 128, D], BF16, kind="ExternalInput").ap()
        yd = nc.dram_tensor("y_out", [HALF, D], BF16, kind="ExternalOutput").ap()
        need_dummy_io = False
    vw4d = nc.dram_tensor("vw4", [D, 4], BF16, kind="ExternalInput").ap()
    w3d = nc.dram_tensor("w3", [3, D], BF16, kind="ExternalInput").ap()
    identd = nc.dram_tensor("ident", [128, 128], BF16, kind="ExternalInput").ap()
    t128d = nc.dram_tensor("t128", [128, 128], F32, kind="ExternalInput").ap()
    frowd = nc.dram_tensor("frow", [128, 1], F32, kind="ExternalInput").ap()
    apow1d = nc.dram_tensor("apow1", [1, 128], F32, kind="ExternalInput").ap()
    alrowd = nc.dram_tensor("alrow", [1, nt], F32, kind="ExternalInput").ap()
    colsd = nc.dram_tensor("cols3", [128, 3], F32, kind="ExternalInput").ap()

    with tile.TileContext(nc) as tc:
        with (
            tc.tile_pool(name="xpool", bufs=1) as xpool,
            tc.tile_pool(name="work", bufs=4) as work,
            tc.tile_pool(name="sq", bufs=3) as sqp,
            tc.tile_pool(name="small", bufs=1) as small,
            tc.tile_pool(name="cst", bufs=1) as cst,
            tc.tile_pool(name="ps", bufs=1, space="PSUM") as psp,
        ):
            # ---- constants (ACT queue keeps SP free for x in-DMAs) ----
            vw4 = cst.tile([128, 8, 4], BF16, name="vw4")
            w3b = cst.tile([3, D], BF16, name="w3b")
            ident = cst.tile([128, 128], BF16, name="ident")
            t128 = cst.tile([128, 128], F32, name="t128")
            frow = cst.tile([128, 1], F32, name="frow")
            apow1 = cst.tile([1, 128], F32, name="apow1")
            alrow = cst.tile([1, nt], F32, name="alrow")
            cols3 = cst.tile([128, 3], F32, name="cols3")
            # ident+cols3 first on SP (needed immediately, tiny); the bulkier
            # constants ride the idle Pool queue (SWDGE) ordered by first use,
            # so neither the ACT queue (squares) nor x in-DMAs are delayed.
            nc.sync.dma_start(ident[:], identd[:])
            nc.sync.dma_start(cols3[:], colsd[:])
            nc.gpsimd.dma_start(vw4[:], vw4d.rearrange("(k p) q -> p k q", p=128))
            nc.gpsimd.dma_start(t128[:], t128d[:])
            nc.gpsimd.dma_start(frow[:], frowd[:])
            nc.gpsimd.dma_start(apow1[:], apow1d[:])
            nc.gpsimd.dma_start(alrow[:], alrowd[:])
            nc.gpsimd.dma_start(w3b[:], w3d[:])

            # ---- per-token arrays (fp32 scalar pipeline, [128tok, nt]) ----
            d4 = small.tile([128, nt, 4], F32, name="d4")
            s0 = small.tile([128, nt], F32, name="s0")
            vms = small.tile([128, nt], F32, name="vms")
            rstd1 = small.tile([128, nt], F32, name="rstd1")
            rstd2 = small.tile([128, nt], F32, name="rstd2")
            hgb = small.tile([128, nt, 3], BF16, name="hgb")
            u = small.tile([128, nt], F32, name="u")
            scr1 = small.tile([128, nt], F32, name="scr1")
            scr2 = small.tile([128, nt], F32, name="scr2")
            pq1 = small.tile([128, nt], F32, name="pq1")
            pq2 = small.tile([128, nt], F32, name="pq2")
            pms = small.tile([128, nt], F32, name="pms")
            srow = small.tile([1, nt], F32, name="srow")
            crow = small.tile([1, nt], F32, name="crow")
            epsT = small.tile([128, nt], F32, name="epsT")
            onesT = small.tile([128, nt], F32, name="onesT")
            cwwT = small.tile([128, nt], F32, name="cwwT")
            c0T = small.tile([128, nt], F32, name="c0T")
            c1T = small.tile([128, nt], F32, name="c1T")
            nc.vector.memset(epsT[:], float(EPS))
            nc.vector.memset(onesT[:], 1.0)
            nc.vector.scalar_tensor_tensor(cwwT[:], onesT[:], cols3[:, 0:1],
                                           epsT[:], OP.mult, OP.bypass)
            nc.vector.scalar_tensor_tensor(c0T[:], onesT[:], cols3[:, 1:2],
                                           epsT[:], OP.mult, OP.bypass)
            nc.vector.scalar_tensor_tensor(c1T[:], onesT[:], cols3[:, 2:3],
                                           epsT[:], OP.mult, OP.bypass)
            if need_dummy_io:
                dum = small.tile([128, 4], F32, name="dum")
                nc.sync.dma_start(dum[:], dummy_in[:])
                nc.sync.dma_start(dummy_out[:], dum[:])
            ct_sb = small.tile([3, nt * 128], BF16, name="ct_sb")

            def body():
                x0 = xpool.tile([128, D], BF16, tag="x0", name="x0")
                xpairs = [xpool.tile([128, 2, D], BF16, tag=f"xp{j}", name=f"xp{j}")
                          for j in range(MAIN_TILES // 2)]

                def xview(i):
                    if i == 0:
                        return x0[:]
                    j, s = (i - 1) // 2, (i - 1) % 2
                    return xpairs[j][:, s, :]

                def a_load(g):
                    b0, b1 = groups[g]
                    for i in range(b0, b1):
                        if i == 0:
                            nc.sync.dma_start(x0[:], xd[0:128, :])
                        elif (i - 1) % 2 == 0:
                            j = (i - 1) // 2
                            nc.sync.dma_start(
                                xpairs[j][:],
                                xd[i * 128:(i + 2) * 128, :]
                                .rearrange("(b p) d -> p b d", p=128))

                pair_sbs = {}

                def a_main(g):
                    b0, b1 = groups[g]
                    glen = b1 - b0
                    for i in range(b0, b1):
                        sq = sqp.tile([128, D], BF16, tag="sq", name=f"sq{i}",
                                      bufs=3)
                        if SQ_DVE_N and i % max(1, 33 // SQ_DVE_N) == 5 % max(1, 33 // SQ_DVE_N):
                            nc.vector.scalar_tensor_tensor(
                                sq[:], xview(i), 1.0, xview(i),
                                OP.mult, OP.mult, accum_out=s0[:, i:i + 1])
                        else:
                            nc.scalar.activation(sq[:], xview(i), AF.Square,
                                                 accum_out=s0[:, i:i + 1])
                    for p in range(4):
                        pair_ps = psp.tile([128, D], BF16, tag="pair_ps",
                                           name=f"pair_ps{g}_{p}", bufs=2)
                        for sl in range(2):
                            k = 2 * p + sl
                            for gi, i in enumerate(range(b0, b1)):
                                nc.tensor.transpose(
                                    pair_ps[:, sl * 512 + gi * 128:
                                            sl * 512 + (gi + 1) * 128],
                                    xview(i)[:, k * 128:(k + 1) * 128],
                                    ident[:])
                        pair_sb = work.tile([128, D], BF16, tag="pair_sb",
                                            name=f"pair_sb{g}_{p}", bufs=8)
                        pair_sbs[(g, p)] = pair_sb
                        cpi = g * 4 + p
                        if ((cpi + 1) * PAIRCP_ACT_N) // 40 > (cpi * PAIRCP_ACT_N) // 40:
                            nc.scalar.copy(pair_sb[:].bitcast(I32),
                                           pair_ps[:].bitcast(I32))
                        else:
                            nc.vector.tensor_copy(pair_sb[:], pair_ps[:])

                def d4f_tile(idx):
                    # shared PSUM bank: d4 dots [:,0:16], scan row-sum
                    # [0:1,16:24], ct transposes [0:3,24:280].bitcast(BF16).
                    # Accumulation groups never overlap in time (all issued
                    # from the in-order PE queue); two buffers alternate by
                    # index parity (deterministic via the name).
                    return psp.tile([128, 284], F32, tag="d4f",
                                    name=f"d4f{idx % 2}", bufs=2)

                def a_dots(g):
                    b0, b1 = groups[g]
                    glen = b1 - b0
                    d4_ps = d4f_tile(g)
                    KCH = int(os.environ.get("DOTS_KCHUNKS", "8"))
                    for gi in range(glen):
                        for k in range(KCH):
                            p, sl = k // 2, k % 2
                            lhsT = pair_sbs[(g, p)][:, sl * 512 + gi * 128:
                                                    sl * 512 + (gi + 1) * 128]
                            nc.tensor.matmul(d4_ps[:, gi * 4:(gi + 1) * 4],
                                             lhsT, vw4[:, k, :],
                                             start=(k == 0), stop=(k == KCH - 1),
                                             skip_group_check=True)
                    nc.vector.tensor_copy(d4[:, b0:b1, :], d4_ps[:, 0:glen * 4])

                def s1_pre(sb):
                    b0, b1 = sblocks[sb][:2]
                    sl = slice(b0, b1)
                    V = nc.vector
                    # rstd1 = rsqrt(v), v = s0/D + eps (bit-trick seed + 1 NR)
                    V.scalar_tensor_tensor(vms[:, sl], s0[:, sl], float(1.0 / D),
                                           epsT[:, sl], OP.mult, OP.add)
                    V.tensor_scalar(scr1[:, sl].bitcast(I32), vms[:, sl].bitcast(I32),
                                    1, 0, OP.logical_shift_right, OP.bitwise_or)
                    V.tensor_scalar(scr2[:, sl].bitcast(I32), scr1[:, sl].bitcast(I32),
                                    -1, MAGIC, OP.mult, OP.add)
                    V.tensor_mul(scr1[:, sl], scr2[:, sl], scr2[:, sl])   # z^2
                    V.tensor_mul(scr1[:, sl], scr1[:, sl], vms[:, sl])    # v z^2
                    V.tensor_scalar(scr1[:, sl], scr1[:, sl], -0.5, 1.5,
                                    OP.mult, OP.add)                      # 1.5-.5vz^2
                    V.tensor_mul(rstd1[:, sl], scr1[:, sl], scr2[:, sl])
                    V.tensor_mul(u[:, sl], d4[:, sl, 0], rstd1[:, sl])

                def s1_rest(sb):
                    b0, b1 = sblocks[sb][:2]
                    bw = b1 - b0
                    sl = slice(b0, b1)
                    V = nc.vector
                    loc_ps = psp.tile([128, bw], F32, tag="loc_ps",
                                      name=f"loc{sb}", bufs=1)
                    f_ps = d4f_tile(sb)[0:1, 16:16 + bw]
                    nc.tensor.matmul(loc_ps[:], t128[:], u[:, sl],
                                     start=True, stop=False)
                    nc.tensor.matmul(f_ps, frow[:], u[:, sl],
                                     start=True, stop=True,
                                     skip_group_check=True)
                    init = 0.0 if sb == 0 else srow[0:1, b0 - 1:b0]
                    V.tensor_tensor_scan(srow[0:1, sl], alrow[0:1, sl], f_ps,
                                         init, OP.mult, OP.add)
                    if sb == 0:
                        V.memset(crow[0:1, 0:1], 0.0)
                        V.tensor_copy(crow[0:1, 1:b1], srow[0:1, 0:b1 - 1])
                        carg = crow[0:1, 0:b1]
                    else:
                        carg = srow[0:1, b0 - 1:b1 - 1]
                    nc.tensor.matmul(loc_ps[:], apow1[:], carg,
                                     start=False, stop=True)
                    V.tensor_copy(hgb[:, sl, 0], loc_ps[:])

                def s2(sb):
                    b0, b1 = sblocks[sb][:2]
                    sl = slice(b0, b1)
                    hb = hgb[:, sl, 0]
                    # rstd2 = rsqrt(ms2), one NR from rstd1 seed (Pool engine,
                    # tensor_tensor/tensor_scalar only; no PSUM access)
                    Gp = nc.gpsimd
                    Gp.tensor_mul(pms[:, sl], hb, cwwT[:, sl])
                    Gp.tensor_add(pms[:, sl], pms[:, sl], d4[:, sl, 1])
                    Gp.tensor_mul(pms[:, sl], pms[:, sl], hb)
                    Gp.tensor_add(pms[:, sl], pms[:, sl], vms[:, sl])      # ms2
                    Gp.tensor_mul(pq1[:, sl], rstd1[:, sl], rstd1[:, sl])
                    Gp.tensor_mul(pq1[:, sl], pq1[:, sl], pms[:, sl])      # ms2 z^2
                    Gp.tensor_scalar(pq1[:, sl], pq1[:, sl], -0.5, 1.5,
                                     OP.mult, OP.add)                      # 1.5-.5w
                    Gp.tensor_mul(rstd2[:, sl], pq1[:, sl], rstd1[:, sl])
                    for cT, dcol, scr in ((c0T, 2, pq1), (c1T, 3, pq2)):
                        Gp.tensor_mul(scr[:, sl], hb, cT[:, sl])
                        Gp.tensor_add(scr[:, sl], scr[:, sl], d4[:, sl, dcol])
                        Gp.tensor_mul(scr[:, sl], scr[:, sl], rstd2[:, sl])

                def s3(sb):
                    b0, b1 = sblocks[sb][:2]
                    sl = slice(b0, b1)
                    nc.scalar.activation(hgb[:, sl, 1], pq1[:, sl],
                                         AF.Gelu_apprx_tanh)
                    nc.scalar.activation(hgb[:, sl, 2], pq2[:, sl],
                                         AF.Gelu_apprx_tanh)

                def b1(g):
                    b0, b1 = groups[g]
                    mains = list(range(max(b0, 1), b1))
                    ml = len(mains)
                    ct_ps = d4f_tile(g)[0:3, 28:28 + 256].bitcast(BF16)
                    for ci, i in enumerate(mains):
                        nc.tensor.transpose(ct_ps[:, ci * 128:(ci + 1) * 128],
                                            hgb[:, i, :], ident[:])
                    nc.vector.tensor_copy(
                        ct_sb[:, mains[0] * 128:(mains[-1] + 1) * 128],
                        ct_ps[:, 0:ml * 128])

                def b2(g):
                    b0, b1 = groups[g]
                    for i in range(max(b0, 1), b1):
                        xt = xview(i)
                        m32 = (i - 1) % 32
                        use_act = ((m32 + 1) * C_ACT_N) // 32 > (m32 * C_ACT_N) // 32
                        for hh in range(2):
                            r3 = psp.tile([128, 512], F32, tag="r3_ps",
                                          name=f"r3_{i}_{hh}", bufs=3)
                            if use_act:
                                nc.tensor.matmul(
                                    r3[:], ident[:],
                                    xt[:, hh * 512:(hh + 1) * 512],
                                    start=True, stop=False)
                            nc.tensor.matmul(
                                r3[:], ct_sb[:, i * 128:(i + 1) * 128],
                                w3b[:, hh * 512:(hh + 1) * 512],
                                start=(not use_act), stop=True,
                                skip_group_check=True)
                            if use_act:
                                nc.scalar.copy(xt[:, hh * 512:(hh + 1) * 512],
                                               r3[:])
                            else:
                                nc.vector.tensor_add(
                                    xt[:, hh * 512:(hh + 1) * 512],
                                    xt[:, hh * 512:(hh + 1) * 512], r3[:])
                        if OUT_QUAD:
                            if (i - 1) % 4 == 3:
                                q = (i - 1) // 4
                                nc.sync.dma_start(
                                    yd[4 * q * 128:(4 * q + 4) * 128, :]
                                    .rearrange("(b p) d -> p b d", p=128),
                                    xpairs[2 * q][:], xpairs[2 * q + 1][:])
                        elif (i - 1) % 2 == 1:   # second tile of its pair
                            j = (i - 1) // 2
                            nc.sync.dma_start(
                                yd[2 * j * 128:(2 * j + 2) * 128, :]
                                .rearrange("(b p) d -> p b d", p=128),
                                xpairs[j][:])

                # sblocks: pairs of groups; S-stages run at sblock width to
                # amortize the small-op overhead (~200ns/op on DVE).
                sblocks = []
                for k in range(0, G, SBLOCK_W):
                    t0 = groups[k][0]
                    t1 = groups[min(k + SBLOCK_W - 1, G - 1)][1]
                    last = min(k + SBLOCK_W - 1, G - 1)
                    sblocks.append((t0, t1, last))
                sched = {}
                for k, (t0, t1, last) in enumerate(sblocks):
                    trig = last + 1 + DOTS_LAG
                    sched.setdefault(trig, {}).setdefault("s1", []).append(k)
                    sched.setdefault(trig + 1, {}).setdefault("s3", []).append(k)
                for g in range(G):
                    k = min(g // SBLOCK_W, len(sblocks) - 1)
                    trig = sblocks[k][2] + 1
                    sched.setdefault(trig + 2, {}).setdefault("b1", []).append(g)
                    sched.setdefault(trig + 3, {}).setdefault("b2", []).append(g)
                n_it = max(sched) + 1
                a_load(0)
                a_load(1)
                a_load(2)
                for i in range(n_it):
                    st = sched.get(i, {})
                    for k in st.get("s1", []):
                        s1_pre(k)
                    if i + 3 < G:
                        a_load(i + 3)
                    if i < G:
                        a_main(i)
                    for k in st.get("s1", []):
                        s1_rest(k)
                        s2(k)
                    for k in st.get("s3", []):
                        s3(k)
                    for g in st.get("b1", []):
                        b1(g)
                    for g in st.get("b2", []):
                        b2(g)
                    if 0 <= i - DOTS_LAG < G:
                        a_dots(i - DOTS_LAG)

            if reps is None:
                body()
            elif reps // unroll == 1:
                for _ in range(unroll):
                    body()
            else:
                assert reps % unroll == 0
                with tc.For_i(0, reps // unroll, 1):
                    for _ in range(unroll):
                        body()
    nc.compile()
    return nc


def host_constants(norm_w, W_in, a_log, W_out, ffn_w1, ffn_w2, nt):
    a = 1.0 / (1.0 + np.exp(-np.float64(a_log[0])))
    Wn = (norm_w * W_in[:, 0]).astype(np.float32)
    Wout_row = W_out[0, :].astype(np.float32)
    W10n = (norm_w * ffn_w1[:, 0]).astype(np.float32)
    W11n = (norm_w * ffn_w1[:, 1]).astype(np.float32)
    # dW column pre-scaled by 2/D so ms2 = v + h*dW2 + h^2*cWW/D
    vw4 = np.stack([Wn, (2.0 / D) * Wout_row, W10n, W11n], axis=1)
    w3 = np.stack([Wout_row, ffn_w2[0, :], ffn_w2[1, :]], axis=0)
    km = np.arange(128)
    expo = km[None, :] - km[:, None]
    t128 = np.where(expo >= 0, a ** np.maximum(expo, 0), 0.0).astype(np.float32)
    frow = (a ** (127 - km)).astype(np.float32).reshape(128, 1)
    apow1 = (a ** (km + 1)).astype(np.float32).reshape(1, 128)
    alrow = np.full((1, nt), a ** 128, dtype=np.float32)
    cWW = np.float32(Wout_row.astype(np.float64) @ Wout_row.astype(np.float64) / D)
    c0 = np.float32(Wout_row.astype(np.float64) @ W10n.astype(np.float64))
    c1 = np.float32(Wout_row.astype(np.float64) @ W11n.astype(np.float64))
    cols3 = np.tile(np.array([cWW, c0, c1], dtype=np.float32), (128, 1))
    return dict(vw4=vw4.astype(BF_NP), w3=w3.astype(BF_NP),
                ident=np.eye(128, dtype=BF_NP), t128=t128,
                frow=frow, apow1=apow1, alrow=alrow, cols3=cols3), a


def pre_tiles_for(a: float) -> int:
    n = int(np.ceil(np.log(1e-9) / (128 * np.log(a))))
    # SBUF keeps all nt tiles resident; cap the prefix (a=sigmoid(a_log) ~ 0.785
    # for the reference inputs -> n=1)
    return min(max(n, 1), 6)


def in_maps_for(x, consts, nt):
    pre = (nt - MAIN_TILES) * 128
    xb = x.astype(BF_NP)
    maps = []
    for c in range(N_CORES):
        b, j = c // 2, c % 2
        if j == 0:
            prefix = np.zeros((pre, D), BF_NP)
        else:
            prefix = np.ascontiguousarray(xb[b, HALF - pre:HALF, :])
        xin = np.concatenate([prefix, xb[b, j * HALF:(j + 1) * HALF, :]], axis=0)
        m = {"x_in": np.ascontiguousarray(xin)}
        m.update(consts)
        maps.append(m)
    return maps


def kernel(x, norm_w, W_in, a_log, W_out, ffn_w1, ffn_w2):
    x = np.asarray(x, dtype=np.float32)
    consts, a = host_constants(np.asarray(norm_w), np.asarray(W_in),
                               np.asarray(a_log), np.asarray(W_out),
                               np.asarray(ffn_w1), np.asarray(ffn_w2), nt=34)
    nt = MAIN_TILES + pre_tiles_for(a)
    consts["alrow"] = np.full((1, nt), np.float64(a) ** 128, dtype=np.float32)

    key = ("plain", nt)
    if key not in _cache:
        _cache[key] = build_program(nt)
    nc = _cache[key]

    res = run_bass_kernel_spmd(nc, in_maps_for(x, consts, nt),
                               core_ids=list(range(N_CORES)))
    out = np.empty((B, S, D), np.float32)
    for c in range(N_CORES):
        b, j = c // 2, c % 2
        out[b, j * HALF:(j + 1) * HALF, :] = res.results[c]["y_out"].astype(np.float32)
    return out


# revision 35
# speedup vs baseline: 1.0514x; 1.0514x over previous
"""Trainium2 Bass kernel for nn_AbsoluteMinimalBlock (rmsnorm -> rank-1 SSM scan -> rmsnorm -> rank-2 FFN).

Math: the whole block is a rank-3 update of x:
    out[t,d] = x[t,d] + h[t]*Wout[d] + g0[t]*W20[d] + g1[t]*W21[d]
  driven by 5 per-token reductions over D:
    d1 = x@(nw*W_in), dW2 = x@(2*Wout/D), dA = x@(nw*w1_0), dB = x@(nw*w1_1),
    S0 = sum(x^2)
  with v = S0/D+eps; rstd1 = rsqrt(v); u = d1*rstd1; h = scan(a, u);
  ms2 = v + h*dW2 + h^2*cWW/D (analytic); rstd2 = rsqrt(ms2);
  p_r = (d_r + h*(Wout.W1r))*rstd2; g_r = gelu_tanh(p_r).

v4 design (group-pipelined, all I/O bf16, DVE fast modes):
  - 9 groups of <=4 tiles, software-pipelined with explicit stage lags so
    every cross-engine dependency is >=1 iteration old (the 4-deep engine
    wait queues otherwise serialize): A(load/square/transpose/copy) ->
    dots(lag1) -> scan(lag2) -> gelu(lag3) -> ctT(lag4) -> w3+add+store(lag5).
  - dots as stationary-x matmuls: d4[tok,4] += pair_sb_chunk.T @ vw4_chunk
    writes the per-token reductions directly in token-partition layout
    (no [4,512] intermediate, no second transpose).
  - rank-3 reconstruct: single K=3 matmul into a BF16 PSUM tile, then one
    DVE tensor_add (bf16 2x mode) + residual; no ACT copies, no identity
    matmuls.
  - ACT does the squares (Square+accum) and small copies via int32-bitcast
    views (halves ACT cycles); DVE does PSUM->SBUF pair copies (bf16 2x)
    and the adds; Pool does the rstd2/gelu-arg scalar chain; per-token
    scalars live in [128tok, ntile] layout so ops are partition-parallel.
  - rsqrt WITHOUT the Scalar-engine Sqrt (sqrt and gelu live in different
    ACT table sets -> 1.3us reload per switch): quake bit-trick seed +
    one Newton step on DVE int ops; rstd2 by one Newton step seeded
    from rstd1. ACT only ever needs {Square, Copy, Gelu_apprx_tanh}.
  - constants DMA on the ACT queue so x in-DMAs start immediately on SP;
    out-DMAs issue from the DVE queue right after the pair's adds.

Sharding: 8 cores = 4 batches x 2 sequence halves; each core prepends a
PRE-tile prefix (zeros for first half / tail of first half for second) so the
scan state is exact (a^128 per tile of decay).
"""
import sys, os
DOTS_LAG = int(os.environ.get("DOTS_LAG", "0"))
SQ_DVE_N = int(os.environ.get("SQ_DVE_N", "3"))      # squares on DVE (of 33)
C_ACT_N = int(os.environ.get("C_ACT_N", "16"))       # residuals on ACT (of 32)
OUT_QUAD = int(os.environ.get("OUT_QUAD", "0"))      # 1 -> 4-tile out DMAs
PAIRCP_ACT_N = int(os.environ.get("PAIRCP_ACT_N", "6"))  # pair copies on ACT (of 40)
SBLOCK_W = int(os.environ.get("SBLOCK_W", "2"))      # groups per scalar sblock
for _p in ("/root/.axon_site/_ro/trn_rl_repo", "/opt/trn_rl_repo"):
    if os.path.isdir(_p) and _p not in sys.path:
        sys.path.append(_p)

import numpy as np
import ml_dtypes
import concourse.bass as bass
import concourse.bacc as bacc
import concourse.mybir as mybir
import concourse.tile as tile
from concourse.bass_utils import run_bass_kernel_spmd

F32 = mybir.dt.float32
I32 = mybir.dt.int32
BF16 = mybir.dt.bfloat16
AF = mybir.ActivationFunctionType
OP = mybir.AluOpType
BF_NP = ml_dtypes.bfloat16

N_CORES = 8
B, S, D = 4, 8192, 1024
HALF = S // 2
MAIN_TILES = HALF // 128      # 32
EPS = 1e-6
MAGIC = 0x5F3759DF

# engine-balance knobs (tuned against the timeline sim + hardware):
# pair copies with (idx % 5) < PAIRCP_ACT_LT run on ACT (int32 view), rest DVE
PAIRCP_ACT_LT = 0
# squares on DVE for tiles with i % 16 == SQ_DVE_MOD, rest ACT
SQ_DVE_MOD = 5  # i % 11 == 5 -> DVE
# residual: tiles with i % 3 == C_ACT_MOD use PE-ident+ACT-copy, rest DVE add
C_ACT_MOD = 1
C_ACT_PERIOD = 2

_cache = {}


def group_bounds(nt):
    """Tile groups aligned to out-DMA pairs: [0,1,2], 4s, then 2s at the
    tail so the pipeline drain processes small groups."""
    gs = [(0, 3)]
    t = 3
    while t + 4 <= nt - 6:
        gs.append((t, t + 4))
        t += 4
    while t < nt:
        gs.append((t, t + 2))
        t += 2
    return gs


def build_program(nt: int, reps=None, internal_io=False, parts=None, unroll=1):
    pre_tiles = nt - MAIN_TILES
    assert pre_tiles == 1, "pair layout assumes exactly one prefix tile"
    groups = group_bounds(nt)
    G = len(groups)

    nc = bacc.Bacc("TRN2", target_bir_lowering=False, debug=False, num_devices=N_CORES)

    if internal_io:
        xd = nc.dram_tensor("x_int", [nt * 128, D], BF16, kind="Internal").ap()
        yd = nc.dram_tensor("y_int", [HALF, D], BF16, kind="Internal").ap()
        dummy_in = nc.dram_tensor("x_in", [128, 4], F32, kind="ExternalInput").ap()
        dummy_out = nc.dram_tensor("y_out", [128, 4], F32, kind="ExternalOutput").ap()
        need_dummy_io = True
    else:
        xd = nc.dram_tensor("x_in", [nt * 128, D], BF16, kind="ExternalInput").ap()
        yd = nc.dram_tensor("y_out", [HALF, D], BF16, kind="ExternalOutput").ap()
        need_dummy_io = False
    vw4d = nc.dram_tensor("vw4", [D, 4], BF16, kind="ExternalInput").ap()
    w3d = nc.dram_tensor("w3", [3, D], BF16, kind="ExternalInput").ap()
    identd = nc.dram_tensor("ident", [128, 128], BF16, kind="ExternalInput").ap()
    t128d = nc.dram_tensor("t128", [128, 128], F32, kind="ExternalInput").ap()
    frowd = nc.dram_tensor("frow", [128, 1], F32, kind="ExternalInput").ap()
    apow1d = nc.dram_tensor("apow1", [1, 128], F32, kind="ExternalInput").ap()
    alrowd = nc.dram_tensor("alrow", [1, nt], F32, kind="ExternalInput").ap()
    colsd = nc.dram_tensor("cols3", [128, 3], F32, kind="ExternalInput").ap()

    with tile.TileContext(nc) as tc:
        with (
            tc.tile_pool(name="xpool", bufs=1) as xpool,
            tc.tile_pool(name="work", bufs=4) as work,
            tc.tile_pool(name="sq", bufs=3) as sqp,
            tc.tile_pool(name="small", bufs=1) as small,
            tc.tile_pool(name="cst", bufs=1) as cst,
            tc.tile_pool(name="ps", bufs=1, space="PSUM") as psp,
        ):
            # ---- constants (ACT queue keeps SP free for x in-DMAs) ----
            vw4 = cst.tile([128, 8, 4], BF16, name="vw4")
            w3b = cst.tile([3, D], BF16, name="w3b")
            ident = cst.tile([128, 128], BF16, name="ident")
            t128 = cst.tile([128, 128], F32, name="t128")
            frow = cst.tile([128, 1], F32, name="frow")
            apow1 = cst.tile([1, 128], F32, name="apow1")
            alrow = cst.tile([1, nt], F32, name="alrow")
            cols3 = cst.tile([128, 3], F32, name="cols3")
            # ident+cols3 first on SP (needed immediately, tiny); the bulkier
            # constants ride the idle Pool queue (SWDGE) ordered by first use,
            # so neither the ACT queue (squares) nor x in-DMAs are delayed.
            nc.sync.dma_start(ident[:], identd[:])
            nc.sync.dma_start(cols3[:], colsd[:])
            nc.gpsimd.dma_start(vw4[:], vw4d.rearrange("(k p) q -> p k q", p=128))
            nc.gpsimd.dma_start(t128[:], t128d[:])
            nc.gpsimd.dma_start(frow[:], frowd[:])
            nc.gpsimd.dma_start(apow1[:], apow1d[:])
            nc.gpsimd.dma_start(alrow[:], alrowd[:])
            nc.gpsimd.dma_start(w3b[:], w3d[:])

            # ---- per-token arrays (fp32 scalar pipeline, [128tok, nt]) ----
            d4 = small.tile([128, nt, 4], F32, name="d4")
            s0 = small.tile([128, nt], F32, name="s0")
            vms = small.tile([128, nt], F32, name="vms")
            rstd1 = small.tile([128, nt], F32, name="rstd1")
            rstd2 = small.tile([128, nt], F32, name="rstd2")
            hgb = small.tile([128, nt, 3], BF16, name="hgb")
            u = small.tile([128, nt], F32, name="u")
            scr1 = small.tile([128, nt], F32, name="scr1")
            scr2 = small.tile([128, nt], F32, name="scr2")
            pq1 = small.tile([128, nt], F32, name="pq1")
            pq2 = small.tile([128, nt], F32, name="pq2")
            pms = small.tile([128, nt], F32, name="pms")
            srow = small.tile([1, nt], F32, name="srow")
            crow = small.tile([1, nt], F32, name="crow")
            epsT = small.tile([128, nt], F32, name="epsT")
            onesT = small.tile([128, nt], F32, name="onesT")
            cwwT = small.tile([128, nt], F32, name="cwwT")
            c0T = small.tile([128, nt], F32, name="c0T")
            c1T = small.tile([128, nt], F32, name="c1T")
            nc.vector.memset(epsT[:], float(EPS))
            nc.vector.memset(onesT[:], 1.0)
            nc.vector.scalar_tensor_tensor(cwwT[:], onesT[:], cols3[:, 0:1],
                                           epsT[:], OP.mult, OP.bypass)
            nc.vector.scalar_tensor_tensor(c0T[:], onesT[:], cols3[:, 1:2],
                                           epsT[:], OP.mult, OP.bypass)
            nc.vector.scalar_tensor_tensor(c1T[:], onesT[:], cols3[:, 2:3],
                                           epsT[:], OP.mult, OP.bypass)
            if need_dummy_io:
                dum = small.tile([128, 4], F32, name="dum")
                nc.sync.dma_start(dum[:], dummy_in[:])
                nc.sync.dma_start(dummy_out[:], dum[:])
            ct_sb = small.tile([3, nt * 128], BF16, name="ct_sb")

            def body():
                x0 = xpool.tile([128, D], BF16, tag="x0", name="x0")
                xpairs = [xpool.tile([128, 2, D], BF16, tag=f"xp{j}", name=f"xp{j}")
                          for j in range(MAIN_TILES // 2)]

                def xview(i):
                    if i == 0:
                        return x0[:]
                    j, s = (i - 1) // 2, (i - 1) % 2
                    return xpairs[j][:, s, :]

                def a_load(g):
                    b0, b1 = groups[g]
                    for i in range(b0, b1):
                        if i == 0:
                            nc.sync.dma_start(x0[:], xd[0:128, :])
                        elif (i - 1) % 2 == 0:
                            j = (i - 1) // 2
                            nc.sync.dma_start(
                                xpairs[j][:],
                                xd[i * 128:(i + 2) * 128, :]
                                .rearrange("(b p) d -> p b d", p=128))

                pair_sbs = {}

                def a_main(g):
                    b0, b1 = groups[g]
                    glen = b1 - b0
                    for i in range(b0, b1):
                        sq = sqp.tile([128, D], BF16, tag="sq", name=f"sq{i}",
                                      bufs=3)
                        if SQ_DVE_N and i % max(1, 33 // SQ_DVE_N) == 5 % max(1, 33 // SQ_DVE_N):
                            nc.vector.scalar_tensor_tensor(
                                sq[:], xview(i), 1.0, xview(i),
                                OP.mult, OP.mult, accum_out=s0[:, i:i + 1])
                        else:
                            nc.scalar.activation(sq[:], xview(i), AF.Square,
                                                 accum_out=s0[:, i:i + 1])
                    for p in range(4):
                        pair_ps = psp.tile([128, D], BF16, tag="pair_ps",
                                           name=f"pair_ps{g}_{p}", bufs=2)
                        for sl in range(2):
                            k = 2 * p + sl
                            for gi, i in enumerate(range(b0, b1)):
                                nc.tensor.transpose(
                                    pair_ps[:, sl * 512 + gi * 128:
                                            sl * 512 + (gi + 1) * 128],
                                    xview(i)[:, k * 128:(k + 1) * 128],
                                    ident[:])
                        pair_sb = work.tile([128, D], BF16, tag="pair_sb",
                                            name=f"pair_sb{g}_{p}", bufs=8)
                        pair_sbs[(g, p)] = pair_sb
                        cpi = g * 4 + p
                        if ((cpi + 1) * PAIRCP_ACT_N) // 40 > (cpi * PAIRCP_ACT_N) // 40:
                            nc.scalar.copy(pair_sb[:].bitcast(I32),
                                           pair_ps[:].bitcast(I32))
                        else:
                            nc.vector.tensor_copy(pair_sb[:], pair_ps[:])

                def d4f_tile(idx):
                    # shared PSUM bank: d4 dots [:,0:16], scan row-sum
                    # [0:1,16:24], ct transposes [0:3,24:280].bitcast(BF16).
                    # Accumulation groups never overlap in time (all issued
                    # from the in-order PE queue); two buffers alternate by
                    # index parity (deterministic via the name).
                    return psp.tile([128, 284], F32, tag="d4f",
                                    name=f"d4f{idx % 2}", bufs=2)

                def a_dots(g):
                    b0, b1 = groups[g]
                    glen = b1 - b0
                    d4_ps = d4f_tile(g)
                    KCH = int(os.environ.get("DOTS_KCHUNKS", "8"))
                    for gi in range(glen):
                        for k in range(KCH):
                            p, sl = k // 2, k % 2
                            lhsT = pair_sbs[(g, p)][:, sl * 512 + gi * 128:
                                                    sl * 512 + (gi + 1) * 128]
                            nc.tensor.matmul(d4_ps[:, gi * 4:(gi + 1) * 4],
                                             lhsT, vw4[:, k, :],
                                             start=(k == 0), stop=(k == KCH - 1),
                                             skip_group_check=True)
                    nc.vector.tensor_copy(d4[:, b0:b1, :], d4_ps[:, 0:glen * 4])

                def s1_pre(sb):
                    b0, b1 = sblocks[sb][:2]
                    sl = slice(b0, b1)
                    V = nc.vector
                    # rstd1 = rsqrt(v), v = s0/D + eps (bit-trick seed + 1 NR)
                    V.scalar_tensor_tensor(vms[:, sl], s0[:, sl], float(1.0 / D),
                                           epsT[:, sl], OP.mult, OP.add)
                    V.tensor_scalar(scr1[:, sl].bitcast(I32), vms[:, sl].bitcast(I32),
                                    1, 0, OP.logical_shift_right, OP.bitwise_or)
                    V.tensor_scalar(scr2[:, sl].bitcast(I32), scr1[:, sl].bitcast(I32),
                                    -1, MAGIC, OP.mult, OP.add)
                    V.tensor_mul(scr1[:, sl], scr2[:, sl], scr2[:, sl])   # z^2
                    V.tensor_mul(scr1[:, sl], scr1[:, sl], vms[:, sl])    # v z^2
                    V.tensor_scalar(scr1[:, sl], scr1[:, sl], -0.5, 1.5,
                                    OP.mult, OP.add)                      # 1.5-.5vz^2
                    V.tensor_mul(rstd1[:, sl], scr1[:, sl], scr2[:, sl])
                    V.tensor_mul(u[:, sl], d4[:, sl, 0], rstd1[:, sl])

                def s1_rest(sb):
                    b0, b1 = sblocks[sb][:2]
                    bw = b1 - b0
                    sl = slice(b0, b1)
                    V = nc.vector
                    loc_ps = psp.tile([128, bw], F32, tag="loc_ps",
                                      name=f"loc{sb}", bufs=1)
                    f_ps = d4f_tile(sb)[0:1, 16:16 + bw]
                    nc.tensor.matmul(loc_ps[:], t128[:], u[:, sl],
                                     start=True, stop=False)
                    nc.tensor.matmul(f_ps, frow[:], u[:, sl],
                                     start=True, stop=True,
                                     skip_group_check=True)
                    init = 0.0 if sb == 0 else srow[0:1, b0 - 1:b0]
                    V.tensor_tensor_scan(srow[0:1, sl], alrow[0:1, sl], f_ps,
                                         init, OP.mult, OP.add)
                    if sb == 0:
                        V.memset(crow[0:1, 0:1], 0.0)
                        V.tensor_copy(crow[0:1, 1:b1], srow[0:1, 0:b1 - 1])
                        carg = crow[0:1, 0:b1]
                    else:
                        carg = srow[0:1, b0 - 1:b1 - 1]
                    nc.tensor.matmul(loc_ps[:], apow1[:], carg,
                                     start=False, stop=True)
                    V.tensor_copy(hgb[:, sl, 0], loc_ps[:])

                def s2(sb):
                    b0, b1 = sblocks[sb][:2]
                    sl = slice(b0, b1)
                    hb = hgb[:, sl, 0]
                    # rstd2 = rsqrt(ms2), one NR from rstd1 seed (Pool engine,
                    # tensor_tensor/tensor_scalar only; no PSUM access)
                    Gp = nc.gpsimd
                    Gp.tensor_mul(pms[:, sl], hb, cwwT[:, sl])
                    Gp.tensor_add(pms[:, sl], pms[:, sl], d4[:, sl, 1])
                    Gp.tensor_mul(pms[:, sl], pms[:, sl], hb)
                    Gp.tensor_add(pms[:, sl], pms[:, sl], vms[:, sl])      # ms2
                    Gp.tensor_mul(pq1[:, sl], rstd1[:, sl], rstd1[:, sl])
                    Gp.tensor_mul(pq1[:, sl], pq1[:, sl], pms[:, sl])      # ms2 z^2
                    Gp.tensor_scalar(pq1[:, sl], pq1[:, sl], -0.5, 1.5,
                                     OP.mult, OP.add)                      # 1.5-.5w
                    Gp.tensor_mul(rstd2[:, sl], pq1[:, sl], rstd1[:, sl])
                    for cT, dcol, scr in ((c0T, 2, pq1), (c1T, 3, pq2)):
                        Gp.tensor_mul(scr[:, sl], hb, cT[:, sl])
                        Gp.tensor_add(scr[:, sl], scr[:, sl], d4[:, sl, dcol])
                        Gp.tensor_mul(scr[:, sl], scr[:, sl], rstd2[:, sl])

                def s3(sb):
                    b0, b1 = sblocks[sb][:2]
                    sl = slice(b0, b1)
                    nc.scalar.activation(hgb[:, sl, 1], pq1[:, sl],
                                         AF.Gelu_apprx_tanh)
                    nc.scalar.activation(hgb[:, sl, 2], pq2[:, sl],
                                         AF.Gelu_apprx_tanh)

                def b1(g):
                    b0, b1 = groups[g]
                    mains = list(range(max(b0, 1), b1))
                    ml = len(mains)
                    ct_ps = d4f_tile(g)[0:3, 28:28 + 256].bitcast(BF16)
                    for ci, i in enumerate(mains):
                        nc.tensor.transpose(ct_ps[:, ci * 128:(ci + 1) * 128],
                                            hgb[:, i, :], ident[:])
                    nc.vector.tensor_copy(
                        ct_sb[:, mains[0] * 128:(mains[-1] + 1) * 128],
                        ct_ps[:, 0:ml * 128])

                def b2(g):
                    b0, b1 = groups[g]
                    for i in range(max(b0, 1), b1):
                        xt = xview(i)
                        m32 = (i - 1) % 32
                        use_act = ((m32 + 1) * C_ACT_N) // 32 > (m32 * C_ACT_N) // 32
                        for hh in range(2):
                            r3 = psp.tile([128, 512], F32, tag="r3_ps",
                                          name=f"r3_{i}_{hh}", bufs=3)
                            if use_act:
                                nc.tensor.matmul(
                                    r3[:], ident[:],
                                    xt[:, hh * 512:(hh + 1) * 512],
                                    start=True, stop=False)
                            nc.tensor.matmul(
                                r3[:], ct_sb[:, i * 128:(i + 1) * 128],
                                w3b[:, hh * 512:(hh + 1) * 512],
                                start=(not use_act), stop=True,
                                skip_group_check=True)
                            if use_act:
                                nc.scalar.copy(xt[:, hh * 512:(hh + 1) * 512],
                                               r3[:])
                            else:
                                nc.vector.tensor_add(
                                    xt[:, hh * 512:(hh + 1) * 512],
                                    xt[:, hh * 512:(hh + 1) * 512], r3[:])
                        if OUT_QUAD:
                            if (i - 1) % 4 == 3:
                                q = (i - 1) // 4
                                nc.sync.dma_start(
                                    yd[4 * q * 128:(4 * q + 4) * 128, :]
                                    .rearrange("(b p) d -> p b d", p=128),
                                    xpairs[2 * q][:], xpairs[2 * q + 1][:])
                        elif (i - 1) % 2 == 1:   # second tile of its pair
                            j = (i - 1) // 2
                            nc.sync.dma_start(
                                yd[2 * j * 128:(2 * j + 2) * 128, :]
                                .rearrange("(b p) d -> p b d", p=128),
                                xpairs[j][:])

                # sblocks: pairs of groups; S-stages run at sblock width to
                # amortize the small-op overhead (~200ns/op on DVE).
                sblocks = []
                for k in range(0, G, SBLOCK_W):
                    t0 = groups[k][0]
                    t1 = groups[min(k + SBLOCK_W - 1, G - 1)][1]
                    last = min(k + SBLOCK_W - 1, G - 1)
                    sblocks.append((t0, t1, last))
                sched = {}
                for k, (t0, t1, last) in enumerate(sblocks):
                    trig = last + 1 + DOTS_LAG
                    sched.setdefault(trig, {}).setdefault("s1", []).append(k)
                    sched.setdefault(trig + 1, {}).setdefault("s3", []).append(k)
                for g in range(G):
                    k = min(g // SBLOCK_W, len(sblocks) - 1)
                    trig = sblocks[k][2] + 1
                    sched.setdefault(trig + 2, {}).setdefault("b1", []).append(g)
                    sched.setdefault(trig + 3, {}).setdefault("b2", []).append(g)
                n_it = max(sched) + 1
                a_load(0)
                a_load(1)
                a_load(2)
                for i in range(n_it):
                    st = sched.get(i, {})
                    for k in st.get("s1", []):
                        s1_pre(k)
                    if i + 3 < G:
                        a_load(i + 3)
                    if i < G:
                        a_main(i)
                    for k in st.get("s1", []):
                        s1_rest(k)
                        s2(k)
                    for k in st.get("s3", []):
                        s3(k)
                    for g in st.get("b1", []):
                        b1(g)
                    for g in st.get("b2", []):
                        b2(g)
                    if 0 <= i - DOTS_LAG < G:
                        a_dots(i - DOTS_LAG)

            if reps is None:
                body()
            elif reps // unroll == 1:
                for _ in range(unroll):
                    body()
            else:
                assert reps % unroll == 0
                with tc.For_i(0, reps // unroll, 1):
                    for _ in range(unroll):
                        body()
    nc.compile()
    return nc


def host_constants(norm_w, W_in, a_log, W_out, ffn_w1, ffn_w2, nt):
    a = 1.0 / (1.0 + np.exp(-np.float64(a_log[0])))
    Wn = (norm_w * W_in[:, 0]).astype(np.float32)
    Wout_row = W_out[0, :].astype(np.float32)
    W10n = (norm_w * ffn_w1[:, 0]).astype(np.float32)
    W11n = (norm_w * ffn_w1[:, 1]).astype(np.float32)
    # dW column pre-scaled by 2/D so ms2 = v + h*dW2 + h^2*cWW/D
    vw4 = np.stack([Wn, (2.0 / D) * Wout_row, W10n, W11n], axis=1)
    w3 = np.stack([Wout_row, ffn_w2[0, :], ffn_w2[1, :]], axis=0)
    km = np.arange(128)
    expo = km[None, :] - km[:, None]
    t128 = np.where(expo >= 0, a ** np.maximum(expo, 0), 0.0).astype(np.float32)
    frow = (a ** (127 - km)).astype(np.float32).reshape(128, 1)
    apow1 = (a ** (km + 1)).astype(np.float32).reshape(1, 128)
    alrow = np.full((1, nt), a ** 128, dtype=np.float32)
    cWW = np.float32(Wout_row.astype(np.float64) @ Wout_row.astype(np.float64) / D)
    c0 = np.float32(Wout_row.astype(np.float64) @ W10n.astype(np.float64))
    c1 = np.float32(Wout_row.astype(np.float64) @ W11n.astype(np.float64))
    cols3 = np.tile(np.array([cWW, c0, c1], dtype=np.float32), (128, 1))
    return dict(vw4=vw4.astype(BF_NP), w3=w3.astype(BF_NP),
                ident=np.eye(128, dtype=BF_NP), t128=t128,
                frow=frow, apow1=apow1, alrow=alrow, cols3=cols3), a


def pre_tiles_for(a: float) -> int:
    n = int(np.ceil(np.log(1e-9) / (128 * np.log(a))))
    # SBUF keeps all nt tiles resident; cap the prefix (a=sigmoid(a_log) ~ 0.785
    # for the reference inputs -> n=1)
    return min(max(n, 1), 6)


def in_maps_for(x, consts, nt):
    pre = (nt - MAIN_TILES) * 128
    xb = x.astype(BF_NP)
    maps = []
    for c in range(N_CORES):
        b, j = c // 2, c % 2
        if j == 0:
            prefix = np.zeros((pre, D), BF_NP)
        else:
            prefix = np.ascontiguousarray(xb[b, HALF - pre:HALF, :])
        xin = np.concatenate([prefix, xb[b, j * HALF:(j + 1) * HALF, :]], axis=0)
        m = {"x_in": np.ascontiguousarray(xin)}
        m.update(consts)
        maps.append(m)
    return maps


def kernel(x, norm_w, W_in, a_log, W_out, ffn_w1, ffn_w2):
    x = np.asarray(x, dtype=np.float32)
    consts, a = host_constants(np.asarray(norm_w), np.asarray(W_in),
                               np.asarray(a_log), np.asarray(W_out),
                               np.asarray(ffn_w1), np.asarray(ffn_w2), nt=34)
    nt = MAIN_TILES + pre_tiles_for(a)
    consts["alrow"] = np.full((1, nt), np.float64(a) ** 128, dtype=np.float32)

    key = ("plain", nt)
    if key not in _cache:
        _cache[key] = build_program(nt)
    nc = _cache[key]

    res = run_bass_kernel_spmd(nc, in_maps_for(x, consts, nt),
                               core_ids=list(range(N_CORES)))
    out = np.empty((B, S, D), np.float32)
    for c in range(N_CORES):
        b, j = c // 2, c % 2
        out[b, j * HALF:(j + 1) * HALF, :] = res.results[c]["y_out"].astype(np.float32)
    return out
